# revision 4
# baseline (speedup 1.0000x reference)
"""Trainium2 Bass kernel for nn_Net_79027398246747 (4-layer binarized MLP).

Strategy:
- Data-parallel over batch: 8 cores x 512 rows each; weights replicated.
- Internally feature-major (h.T layout [F, B]) matching XLA-neuron's schedule.
- Layer 1: fp32 PE matmul, contiguous 128-row K chunks, ascending, single
  PSUM accumulation group -> bitwise-matches the XLA-neuron reference matmul.
- Layers 2-4: activations/weights are exactly +-1 (sign), so fp8e4 DoubleRow
  matmuls give exact integer results (any accumulation order).
- BatchNorm epilogue replicates XLA's fused rounding exactly:
      z = (p - m) * g      (one tensor_scalar: sub, mult)
      y = (z * r) + b      (one tensor_scalar: mult, add)
  with r = rsqrt(v + eps) computed on-device via a tiny jax op (the same
  GPSIMD lowering the reference uses, so bits match).
- sign: (y >= 0) -> {0,1} -> 2*t-1 in fp8 (exactly +-1; matches reference's
  clip/sign/STE chain which provably rounds back to +-1).
- npasses=4 averaging: passes are identical; replicate XLA's CSE'd tail
  (((y+y)+y)+y)*0.25 bitwise on host.
"""

import numpy as np
import ml_dtypes

B, IN, H, OUT = 4096, 3072, 4096, 1000
OUTP = 1024            # padded output features
NCORES = 8
NB = B // NCORES       # 512 batch rows per core
K1 = IN // 128         # 24 K-chunks for layer 1
K2 = H // 128          # 32 K-chunks for layers 2-4
F1 = H // 128          # 32 output feature tiles for layers 1-3
F4 = OUTP // 128       # 8 output feature tiles for layer 4
EPS = np.float32(1e-5)

_CACHE = {}


def _build_bass():
    import concourse.bacc as bacc
    import concourse.mybir as mybir
    from concourse.tile import TileContext

    fp32 = mybir.dt.float32
    fp32r = mybir.dt.float32r
    fp8 = mybir.dt.float8e4
    DR = mybir.MatmulPerfMode.DoubleRow

    nc = bacc.Bacc(trn_type="TRN2")

    xT = nc.dram_tensor("xT", [128, K1, NB], fp32, kind="ExternalInput")
    w1 = nc.dram_tensor("w1", [F1, 128, K1, 128], fp8, kind="ExternalInput")
    w2 = nc.dram_tensor("w2", [F1, 128, K2, 128], fp8, kind="ExternalInput")
    w3 = nc.dram_tensor("w3", [F1, 128, K2, 128], fp8, kind="ExternalInput")
    w4 = nc.dram_tensor("w4", [F4, 128, K2, 128], fp8, kind="ExternalInput")
    # Per-feature BN scalars, [128 partitions, n_feature_tiles], feature = t*128+p
    bn123 = nc.dram_tensor("bn123", [128, 3, 4, F1], fp32, kind="ExternalInput")
    bn4 = nc.dram_tensor("bn4", [128, 4, F4], fp32, kind="ExternalInput")
    out = nc.dram_tensor("out", [OUTP, NB], fp32, kind="ExternalOutput")

    with TileContext(nc) as tc:
        with tc.tile_pool(name="persist", bufs=1) as persist, \
             tc.tile_pool(name="w8pool", bufs=4) as w8pool, \
             tc.tile_pool(name="w32pool", bufs=4) as w32pool, \
             tc.tile_pool(name="zpool", bufs=6) as zpool, \
             tc.tile_pool(name="psum", bufs=8, space="PSUM") as psum:

            xt = persist.tile([128, K1, NB], fp32)
            for k in range(K1):
                nc.sync.dma_start(out=xt[:, k, :], in_=xT[:, k, :])
            bnt = persist.tile([128, 3, 4, F1], fp32)
            nc.sync.dma_start(out=bnt[:], in_=bn123[:])
            bnt4 = persist.tile([128, 4, F4], fp32)
            nc.sync.dma_start(out=bnt4[:], in_=bn4[:])

            hs = [persist.tile([128, F1, NB], fp8, tag=f"h{i}", name=f"h{i}")
                  for i in range(3)]

            def epilogue(ps, li, f, htile, n_ft):
                # z = (p - m) * g ; y = (z * r) + b ; sign -> fp8 +-1
                if li < 3:
                    m = bnt[:, li, 0, f:f + 1]
                    g = bnt[:, li, 1, f:f + 1]
                    r = bnt[:, li, 2, f:f + 1]
                    b = bnt[:, li, 3, f:f + 1]
                else:
                    m = bnt4[:, 0, f:f + 1]
                    g = bnt4[:, 1, f:f + 1]
                    r = bnt4[:, 2, f:f + 1]
                    b = bnt4[:, 3, f:f + 1]
                z = zpool.tile([128, NB], fp32, tag="z")
                nc.vector.tensor_scalar(z[:], ps[:], m, g,
                                        op0=mybir.AluOpType.subtract,
                                        op1=mybir.AluOpType.mult)
                y = zpool.tile([128, NB], fp32, tag="y")
                nc.vector.tensor_scalar(y[:], z[:], r, b,
                                        op0=mybir.AluOpType.mult,
                                        op1=mybir.AluOpType.add)
                if li == 3:
                    nc.sync.dma_start(out=out[f * 128:(f + 1) * 128, :], in_=y[:])
                    return
                t = zpool.tile([128, NB], fp8, tag="t")
                nc.vector.tensor_scalar(t[:], y[:], 0.0, None,
                                        op0=mybir.AluOpType.is_ge)
                nc.scalar.activation(htile[:, f, :], t[:],
                                     mybir.ActivationFunctionType.Copy,
                                     bias=-1.0, scale=2.0)

            # ---- Layer 1: fp32, contiguous K chunks ascending ----
            for f in range(F1):
                w8 = w8pool.tile([128, K1, 128], fp8, tag="w18")
                nc.sync.dma_start(out=w8[:], in_=w1[f])
                wf = w32pool.tile([128, K1, 128], fp32, tag="w1f")
                nc.gpsimd.tensor_copy(wf[:], w8[:])
                ps = psum.tile([128, NB], fp32, tag="ps")
                for k in range(K1):
                    nc.tensor.matmul(ps[:], wf[:, k, :], xt[:, k, :],
                                     start=(k == 0), stop=(k == K1 - 1))
                epilogue(ps, 0, f, hs[0], F1)

            # ---- Layers 2-3: fp8 DoubleRow (exact) ----
            for li, (w, hin, hout) in enumerate([(w2, hs[0], hs[1]),
                                                 (w3, hs[1], hs[2])], start=1):
                for f in range(F1):
                    w8 = w8pool.tile([128, K2, 128], fp8, tag="w8")
                    nc.sync.dma_start(out=w8[:], in_=w[f])
                    ps = psum.tile([128, NB], fp32, tag="ps")
                    for i in range(K2 // 2):
                        nc.tensor.matmul(ps[:], w8[:, 2 * i:2 * i + 2, :],
                                         hin[:, 2 * i:2 * i + 2, :],
                                         start=(i == 0), stop=(i == K2 // 2 - 1),
                                         perf_mode=DR)
                    epilogue(ps, li, f, hout, F1)

            # ---- Layer 4: fp8 DoubleRow + affine only ----
            for f in range(F4):
                w8 = w8pool.tile([128, K2, 128], fp8, tag="w8")
                nc.sync.dma_start(out=w8[:], in_=w4[f])
                ps = psum.tile([128, NB], fp32, tag="ps")
                for i in range(K2 // 2):
                    nc.tensor.matmul(ps[:], w8[:, 2 * i:2 * i + 2, :],
                                     hs[2][:, 2 * i:2 * i + 2, :],
                                     start=(i == 0), stop=(i == K2 // 2 - 1),
                                     perf_mode=DR)
                epilogue(ps, 3, f, None, F4)

    nc.finalize()
    return nc


def _device_rsqrt(v):
    """rsqrt(v + eps) with the same bits as the neuron reference (GPSIMD op)."""
    import jax
    fn = _CACHE.get("rsqrt_fn")
    if fn is None:
        fn = jax.jit(lambda t: jax.lax.rsqrt(t + EPS))
        _CACHE["rsqrt_fn"] = fn
    return np.asarray(fn(v.astype(np.float32)))


def _sign8(w):
    return np.where(w >= 0, 1, -1).astype(ml_dtypes.float8_e4m3)


def _prep_w(ws, n_k, n_f):
    # ws: [F_total, K_total] +-1 fp8 -> [n_f, 128, n_k, 128]:
    # out[f, p, k, j] = ws[f*128+j, k*128+p]
    Ft, Kt = ws.shape
    a = ws.reshape(n_f, 128, n_k, 128)          # [f, j, k, p]
    return np.ascontiguousarray(a.transpose(0, 3, 2, 1))


def _prep_bn(bn, n_f):
    # bn: [4, F] (g, b, m, v) -> [128, 4, n_f] with rows (m, g, r, b)
    g, b, m, v = bn[0], bn[1], bn[2], bn[3]
    r = _device_rsqrt(v)
    stack = np.stack([m, g, r, b])              # [4, F]
    a = stack.reshape(4, n_f, 128)              # [c, f, p]
    return np.ascontiguousarray(a.transpose(2, 0, 1)).astype(np.float32)


def kernel(x, w1, w2, w3, w4, bn1, bn2, bn3, bn4):
    from concourse.bass_utils import run_bass_kernel_spmd

    x = np.asarray(x, dtype=np.float32)
    nc = _CACHE.get("nc")
    if nc is None:
        nc = _build_bass()
        _CACHE["nc"] = nc

    w1p = _prep_w(_sign8(np.asarray(w1)), K1, F1)
    w2p = _prep_w(_sign8(np.asarray(w2)), K2, F1)
    w3p = _prep_w(_sign8(np.asarray(w3)), K2, F1)
    w4s = _sign8(np.asarray(w4))
    w4pad = np.zeros((OUTP, H), dtype=ml_dtypes.float8_e4m3)
    w4pad[:OUT] = w4s
    w4p = _prep_w(w4pad, K2, F4)

    b123 = np.stack([_prep_bn(np.asarray(bn1), F1),
                     _prep_bn(np.asarray(bn2), F1),
                     _prep_bn(np.asarray(bn3), F1)], axis=1)  # [128, 3, 4, F1]
    b123 = np.ascontiguousarray(b123)
    bn4pad = np.zeros((4, OUTP), dtype=np.float32)
    bn4pad[:, :OUT] = np.asarray(bn4)
    bn4pad[3, OUT:] = 1.0
    b4 = _prep_bn(bn4pad, F4)

    in_maps = []
    for c in range(NCORES):
        xs = x[c * NB:(c + 1) * NB]                     # [512, 3072]
        # xT layout [128, K1, NB]: [p, k, n] = x[n, k*128+p]
        xt = np.ascontiguousarray(
            xs.reshape(NB, K1, 128).transpose(2, 1, 0))
        in_maps.append({"xT": xt, "w1": w1p, "w2": w2p, "w3": w3p,
                        "w4": w4p, "bn123": b123, "bn4": b4})

    import os
    trace = bool(os.environ.get("BNN_TRACE"))
    res = run_bass_kernel_spmd(nc, in_maps, core_ids=list(range(NCORES)),
                               trace=trace)
    if trace:
        _CACHE["last_exec_time_ns"] = res.exec_time_ns
        _CACHE["last_profile"] = res.profile_json

    # Gather: out [OUTP, NB] feature-major -> [B, OUT]
    y = np.empty((B, OUT), dtype=np.float32)
    for c in range(NCORES):
        y[c * NB:(c + 1) * NB] = res.results[c]["out"][:OUT, :].T

    _CACHE["last_y"] = y
    # npasses tail, replicating XLA's CSE'd graph bitwise:
    acc = y + y
    acc = acc + y
    acc = acc + y
    return acc * np.float32(0.25)



# revision 12
# speedup vs baseline: 1.6655x; 1.6655x over previous
"""Trainium2 Bass kernel for nn_Net_79027398246747 (4-layer binarized MLP).

Strategy:
- Data-parallel over batch: 8 cores x 512 rows each; weights replicated.
- Internally feature-major (h.T layout [F, B]) matching XLA-neuron's schedule.
- Layer 1: x is split exactly as x = hi + lo with hi = RNE12(x) (tf32
  rounding) and lo = x - hi; both halves are exactly fp32r-representable, so
  two fp32r matmuls per K chunk (lo then hi, single PSUM accumulation group,
  ascending chunks) compute the full-precision product at 1 cycle/row per
  pass (2 total) instead of fp32's 4 cycles/row. This is ulp-equivalent to
  the XLA-neuron fp32 matmul (not bitwise: measured <= a-few-ulp path
  differences; end-to-end this perturbs O(1) rows of the output).
- Layers 2-4: activations/weights are exactly +-1 (sign), so fp8e4 DoubleRow
  matmuls give exact integer results (any accumulation order).
- BatchNorm epilogue replicates XLA's fused rounding exactly:
      z = (p - m) * g      (one tensor_scalar: sub, mult)
      y = (z * r) + b      (one tensor_scalar: mult, add)
  with r = rsqrt(v + eps) computed on-device via a tiny jax op (the same
  GPSIMD lowering the reference uses, so bits match).
- sign: (y >= 0) -> {0,1} -> 2*t-1 in fp8 (exactly +-1; matches reference's
  clip/sign/STE chain which provably rounds back to +-1).
- npasses=4 averaging: passes are identical; replicate XLA's CSE'd tail
  (((y+y)+y)+y)*0.25 bitwise on host.
"""

import numpy as np
import ml_dtypes

B, IN, H, OUT = 4096, 3072, 4096, 1000
OUTP = 1024            # padded output features
NCORES = 8
NB = B // NCORES       # 512 batch rows per core
K1 = IN // 128         # 24 K-chunks for layer 1
K2 = H // 128          # 32 K-chunks for layers 2-4
F1 = H // 128          # 32 output feature tiles for layers 1-3
F4 = OUTP // 128       # 8 output feature tiles for layer 4
EPS = np.float32(1e-5)

_CACHE = {}


def _build_bass():
    import concourse.bacc as bacc
    import concourse.mybir as mybir
    from concourse.tile import TileContext

    fp32 = mybir.dt.float32
    fp32r = mybir.dt.float32r
    fp8 = mybir.dt.float8e4
    DR = mybir.MatmulPerfMode.DoubleRow

    nc = bacc.Bacc(trn_type="TRN2")

    xH = nc.dram_tensor("xH", [128, K1, NB], fp32r, kind="ExternalInput")
    xL = nc.dram_tensor("xL", [128, K1, NB], fp32r, kind="ExternalInput")
    w1 = nc.dram_tensor("w1", [F1, 128, K1, 128], fp8, kind="ExternalInput")
    w2 = nc.dram_tensor("w2", [F1, 128, K2, 128], fp8, kind="ExternalInput")
    w3 = nc.dram_tensor("w3", [F1, 128, K2, 128], fp8, kind="ExternalInput")
    w4 = nc.dram_tensor("w4", [F4, 128, K2, 128], fp8, kind="ExternalInput")
    # Per-feature BN scalars, [128 partitions, n_feature_tiles], feature = t*128+p
    bn123 = nc.dram_tensor("bn123", [128, 3, 4, F1], fp32, kind="ExternalInput")
    bn4 = nc.dram_tensor("bn4", [128, 4, F4], fp32, kind="ExternalInput")
    out = nc.dram_tensor("out", [OUTP, NB], fp32, kind="ExternalOutput")

    with TileContext(nc) as tc:
        with tc.tile_pool(name="persist", bufs=1) as persist, \
             tc.tile_pool(name="w8pool", bufs=3) as w8pool, \
             tc.tile_pool(name="w32pool", bufs=2) as w32pool, \
             tc.tile_pool(name="zpool", bufs=3) as zpool, \
             tc.tile_pool(name="psum", bufs=8, space="PSUM") as psum:

            # Prefetch the first two weight tiles ahead of the (large) x DMA
            # so the PE can start as soon as chunk 0 of x lands.
            w8_pre = []
            for f in range(2):
                w8 = w8pool.tile([128, K1, 128], fp8, tag="w18")
                nc.sync.dma_start(out=w8[:], in_=w1[f])
                w8_pre.append(w8)

            xh = persist.tile([128, K1, NB], fp32r, name="xh")
            xl = persist.tile([128, K1, NB], fp32r, name="xl")
            for k in range(K1):
                nc.sync.dma_start(out=xl[:, k, :], in_=xL[:, k, :])
                nc.sync.dma_start(out=xh[:, k, :], in_=xH[:, k, :])
            bnt = persist.tile([128, 3, 4, F1], fp32)
            nc.sync.dma_start(out=bnt[:], in_=bn123[:])
            bnt4 = persist.tile([128, 4, F4], fp32)
            nc.sync.dma_start(out=bnt4[:], in_=bn4[:])

            hs = [persist.tile([128, F1, NB], fp8, tag=f"h{i}", name=f"h{i}")
                  for i in range(3)]

            def epilogue(ps, li, f, htile, n_ft):
                # z = (p - m) * g ; y = (z * r) + b ; sign -> fp8 +-1
                if li < 3:
                    m = bnt[:, li, 0, f:f + 1]
                    g = bnt[:, li, 1, f:f + 1]
                    r = bnt[:, li, 2, f:f + 1]
                    b = bnt[:, li, 3, f:f + 1]
                else:
                    m = bnt4[:, 0, f:f + 1]
                    g = bnt4[:, 1, f:f + 1]
                    r = bnt4[:, 2, f:f + 1]
                    b = bnt4[:, 3, f:f + 1]
                z = zpool.tile([128, NB], fp32, tag="z")
                nc.vector.tensor_scalar(z[:], ps[:], m, g,
                                        op0=mybir.AluOpType.subtract,
                                        op1=mybir.AluOpType.mult)
                y = zpool.tile([128, NB], fp32, tag="y")
                nc.vector.tensor_scalar(y[:], z[:], r, b,
                                        op0=mybir.AluOpType.mult,
                                        op1=mybir.AluOpType.add)
                if li == 3:
                    nc.sync.dma_start(out=out[f * 128:(f + 1) * 128, :], in_=y[:])
                    return
                t = zpool.tile([128, NB], fp8, tag="t")
                nc.vector.tensor_scalar(t[:], y[:], 0.0, None,
                                        op0=mybir.AluOpType.is_ge)
                nc.scalar.activation(htile[:, f, :], t[:],
                                     mybir.ActivationFunctionType.Copy,
                                     bias=-1.0, scale=2.0)

            # ---- Layer 1: two fp32r passes (lo, hi) per contiguous K chunk,
            #      ascending, one PSUM accumulation group ----
            for f in range(F1):
                if f < 2:
                    w8 = w8_pre[f]
                else:
                    w8 = w8pool.tile([128, K1, 128], fp8, tag="w18")
                    nc.sync.dma_start(out=w8[:], in_=w1[f])
                wf = w32pool.tile([128, K1, 128], fp32, tag="w1f")
                nc.gpsimd.tensor_copy(wf[:].bitcast(fp32r), w8[:])
                ps = psum.tile([128, NB], fp32, tag="ps")
                for k in range(K1):
                    nc.tensor.matmul(ps[:], wf[:, k, :].bitcast(fp32r),
                                     xl[:, k, :],
                                     start=(k == 0), stop=False)
                    nc.tensor.matmul(ps[:], wf[:, k, :].bitcast(fp32r),
                                     xh[:, k, :],
                                     start=False, stop=(k == K1 - 1))
                epilogue(ps, 0, f, hs[0], F1)

            # ---- Layers 2-3: fp8 DoubleRow (exact) ----
            for li, (w, hin, hout) in enumerate([(w2, hs[0], hs[1]),
                                                 (w3, hs[1], hs[2])], start=1):
                for f in range(F1):
                    w8 = w8pool.tile([128, K2, 128], fp8, tag="w8")
                    nc.sync.dma_start(out=w8[:], in_=w[f])
                    ps = psum.tile([128, NB], fp32, tag="ps")
                    for i in range(K2 // 2):
                        nc.tensor.matmul(ps[:], w8[:, 2 * i:2 * i + 2, :],
                                         hin[:, 2 * i:2 * i + 2, :],
                                         start=(i == 0), stop=(i == K2 // 2 - 1),
                                         perf_mode=DR)
                    epilogue(ps, li, f, hout, F1)

            # ---- Layer 4: fp8 DoubleRow + affine only ----
            for f in range(F4):
                w8 = w8pool.tile([128, K2, 128], fp8, tag="w8")
                nc.sync.dma_start(out=w8[:], in_=w4[f])
                ps = psum.tile([128, NB], fp32, tag="ps")
                for i in range(K2 // 2):
                    nc.tensor.matmul(ps[:], w8[:, 2 * i:2 * i + 2, :],
                                     hs[2][:, 2 * i:2 * i + 2, :],
                                     start=(i == 0), stop=(i == K2 // 2 - 1),
                                     perf_mode=DR)
                epilogue(ps, 3, f, None, F4)

    nc.finalize()
    return nc


def _device_rsqrt(v):
    """rsqrt(v + eps) with the same bits as the neuron reference (GPSIMD op)."""
    import jax
    fn = _CACHE.get("rsqrt_fn")
    if fn is None:
        fn = jax.jit(lambda t: jax.lax.rsqrt(t + EPS))
        _CACHE["rsqrt_fn"] = fn
    return np.asarray(fn(v.astype(np.float32)))


def _sign8(w):
    return np.where(w >= 0, 1, -1).astype(ml_dtypes.float8_e4m3)


def _rne12(x):
    """Round fp32 to a 12-bit significand (fp32r/tf32), round-to-nearest-even."""
    xb = x.view(np.uint32).astype(np.uint64)
    rb = ((xb + 0xFFF + ((xb >> 13) & 1)) & 0xFFFFE000).astype(np.uint32)
    return rb.view(np.float32).reshape(x.shape)


def _prep_w(ws, n_k, n_f):
    # ws: [F_total, K_total] +-1 fp8 -> [n_f, 128, n_k, 128]:
    # out[f, p, k, j] = ws[f*128+j, k*128+p]
    Ft, Kt = ws.shape
    a = ws.reshape(n_f, 128, n_k, 128)          # [f, j, k, p]
    return np.ascontiguousarray(a.transpose(0, 3, 2, 1))


def _prep_bn(bn, n_f):
    # bn: [4, F] (g, b, m, v) -> [128, 4, n_f] with rows (m, g, r, b)
    g, b, m, v = bn[0], bn[1], bn[2], bn[3]
    r = _device_rsqrt(v)
    stack = np.stack([m, g, r, b])              # [4, F]
    a = stack.reshape(4, n_f, 128)              # [c, f, p]
    return np.ascontiguousarray(a.transpose(2, 0, 1)).astype(np.float32)


def kernel(x, w1, w2, w3, w4, bn1, bn2, bn3, bn4):
    from concourse.bass_utils import run_bass_kernel_spmd

    x = np.asarray(x, dtype=np.float32)
    nc = _CACHE.get("nc")
    if nc is None:
        nc = _build_bass()
        _CACHE["nc"] = nc

    w1p = _prep_w(_sign8(np.asarray(w1)), K1, F1)
    w2p = _prep_w(_sign8(np.asarray(w2)), K2, F1)
    w3p = _prep_w(_sign8(np.asarray(w3)), K2, F1)
    w4s = _sign8(np.asarray(w4))
    w4pad = np.zeros((OUTP, H), dtype=ml_dtypes.float8_e4m3)
    w4pad[:OUT] = w4s
    w4p = _prep_w(w4pad, K2, F4)

    b123 = np.stack([_prep_bn(np.asarray(bn1), F1),
                     _prep_bn(np.asarray(bn2), F1),
                     _prep_bn(np.asarray(bn3), F1)], axis=1)  # [128, 3, 4, F1]
    b123 = np.ascontiguousarray(b123)
    bn4pad = np.zeros((4, OUTP), dtype=np.float32)
    bn4pad[:, :OUT] = np.asarray(bn4)
    bn4pad[3, OUT:] = 1.0
    b4 = _prep_bn(bn4pad, F4)

    xhi = _rne12(x)
    xlo = (x - xhi).astype(np.float32)

    in_maps = []
    for c in range(NCORES):
        # x layout [128, K1, NB]: [p, k, n] = x[n, k*128+p]
        def lay(a):
            s = a[c * NB:(c + 1) * NB]                  # [512, 3072]
            return np.ascontiguousarray(
                s.reshape(NB, K1, 128).transpose(2, 1, 0))
        in_maps.append({"xH": lay(xhi), "xL": lay(xlo), "w1": w1p,
                        "w2": w2p, "w3": w3p, "w4": w4p,
                        "bn123": b123, "bn4": b4})

    import os
    trace = bool(os.environ.get("BNN_TRACE"))
    res = run_bass_kernel_spmd(nc, in_maps, core_ids=list(range(NCORES)),
                               trace=trace)
    if trace:
        _CACHE["last_exec_time_ns"] = res.exec_time_ns
        _CACHE["last_profile"] = res.profile_json

    # Gather: out [OUTP, NB] feature-major -> [B, OUT]
    y = np.empty((B, OUT), dtype=np.float32)
    for c in range(NCORES):
        y[c * NB:(c + 1) * NB] = res.results[c]["out"][:OUT, :].T

    _CACHE["last_y"] = y
    # npasses tail, replicating XLA's CSE'd graph bitwise:
    acc = y + y
    acc = acc + y
    acc = acc + y
    return acc * np.float32(0.25)



# revision 20
# speedup vs baseline: 1.8142x; 1.0893x over previous
"""Trainium2 Bass kernel for nn_Net_79027398246747 (4-layer binarized MLP).

Strategy:
- Data-parallel over batch: 8 cores x 512 rows each; weights replicated.
- Internally feature-major (h.T layout [F, B]) matching XLA-neuron's schedule.
- Layer 1: x is split exactly as x = hi + lo with hi = RNE12(x) (tf32
  rounding) and lo = x - hi; both halves are exactly fp32r-representable, so
  two fp32r matmuls per K chunk (lo then hi, single PSUM accumulation group,
  ascending chunks) compute the full-precision product at 1 cycle/row per
  pass (2 total) instead of fp32's 4 cycles/row. This is ulp-equivalent to
  the XLA-neuron fp32 matmul (not bitwise: measured <= a-few-ulp path
  differences; end-to-end this perturbs O(1) rows of the output).
- Layers 2-4: activations/weights are exactly +-1 (sign), so fp8e4 DoubleRow
  matmuls give exact integer results (any accumulation order).
- BatchNorm epilogue replicates XLA's fused rounding exactly:
      z = (p - m) * g      (one tensor_scalar: sub, mult)
      y = (z * r) + b      (one tensor_scalar: mult, add)
  with r = rsqrt(v + eps) computed on-device via a tiny jax op (the same
  GPSIMD lowering the reference uses, so bits match).
- sign: (y >= 0) -> {0,1} -> 2*t-1 in fp8 (exactly +-1; matches reference's
  clip/sign/STE chain which provably rounds back to +-1).
- npasses=4 averaging: passes are identical; replicate XLA's CSE'd tail
  (((y+y)+y)+y)*0.25 bitwise on host.
"""

import numpy as np
import ml_dtypes

B, IN, H, OUT = 4096, 3072, 4096, 1000
OUTP = 1024            # padded output features
NCORES = 8
NB = B // NCORES       # 512 batch rows per core
K1 = IN // 128         # 24 K-chunks for layer 1
K2 = H // 128          # 32 K-chunks for layers 2-4
F1 = H // 128          # 32 output feature tiles for layers 1-3
F4 = OUTP // 128       # 8 output feature tiles for layer 4
EPS = np.float32(1e-5)

_CACHE = {}


def _build_bass():
    import concourse.bacc as bacc
    import concourse.mybir as mybir
    from concourse.tile import TileContext

    fp32 = mybir.dt.float32
    fp32r = mybir.dt.float32r
    fp8 = mybir.dt.float8e4
    DR = mybir.MatmulPerfMode.DoubleRow

    nc = bacc.Bacc(trn_type="TRN2")

    xT = nc.dram_tensor("xT", [128, K1, NB], fp32, kind="ExternalInput")
    w1 = nc.dram_tensor("w1", [F1, 128, K1, 128], fp8, kind="ExternalInput")
    w2 = nc.dram_tensor("w2", [F1, 128, K2, 128], fp8, kind="ExternalInput")
    w3 = nc.dram_tensor("w3", [F1, 128, K2, 128], fp8, kind="ExternalInput")
    w4 = nc.dram_tensor("w4", [F4, 128, K2, 128], fp8, kind="ExternalInput")
    # Per-feature BN scalars, [128 partitions, n_feature_tiles], feature = t*128+p
    bn123 = nc.dram_tensor("bn123", [128, 3, 4, F1], fp32, kind="ExternalInput")
    bn4 = nc.dram_tensor("bn4", [128, 4, F4], fp32, kind="ExternalInput")
    out = nc.dram_tensor("out", [OUTP, NB], fp32, kind="ExternalOutput")

    with TileContext(nc) as tc:
        with tc.tile_pool(name="persist", bufs=1) as persist, \
             tc.tile_pool(name="w8pool", bufs=4) as w8pool, \
             tc.tile_pool(name="w32pool", bufs=2) as w32pool, \
             tc.tile_pool(name="zpool", bufs=3) as zpool, \
             tc.tile_pool(name="hpool", bufs=2) as hpool, \
             tc.tile_pool(name="psum", bufs=8, space="PSUM") as psum:

            # Prefetch the first weight tile ahead of the (large) x DMA so
            # the PE can start as soon as chunk 0 of x lands.
            w8_f0 = w8pool.tile([128, K1, 128], fp8, tag="w18")
            nc.sync.dma_start(out=w8_f0[:], in_=w1[0])

            # x arrives once as fp32; the exact split x = hi + lo is built
            # on-chip per chunk: hi = fp32r-rounding copy (RNE to 12-bit
            # significand), lo = x - hi computed in place (exactly
            # fp32r-representable).
            xt = persist.tile([128, K1, NB], fp32, name="xt")
            xh = persist.tile([128, K1, NB], fp32r, name="xh")
            for k in range(K1):
                nc.sync.dma_start(out=xt[:, k, :], in_=xT[:, k, :])
                nc.vector.tensor_copy(xh[:, k, :], xt[:, k, :])
                nc.vector.tensor_tensor(xt[:, k, :].bitcast(fp32r),
                                        xt[:, k, :], xh[:, k, :].bitcast(fp32),
                                        op=mybir.AluOpType.subtract)
            xl = xt  # after the in-place subtract, xt holds the lo half
            bnt = persist.tile([128, 3, 4, F1], fp32)
            nc.sync.dma_start(out=bnt[:], in_=bn123[:])
            bnt4 = persist.tile([128, 4, F4], fp32)
            nc.sync.dma_start(out=bnt4[:], in_=bn4[:])

            # h1 and h3 share a buffer (bufs=2 ring on one tag): h1 is fully
            # consumed by layer 2 before layer 3's epilogue writes h3.
            hs = [hpool.tile([128, F1, NB], fp8, tag="h", name=f"h{i}")
                  for i in range(3)]

            def epilogue(ps, li, f, htile, n_ft):
                # z = (p - m) * g ; y = (z * r) + b ; sign -> fp8 +-1
                if li < 3:
                    m = bnt[:, li, 0, f:f + 1]
                    g = bnt[:, li, 1, f:f + 1]
                    r = bnt[:, li, 2, f:f + 1]
                    b = bnt[:, li, 3, f:f + 1]
                else:
                    m = bnt4[:, 0, f:f + 1]
                    g = bnt4[:, 1, f:f + 1]
                    r = bnt4[:, 2, f:f + 1]
                    b = bnt4[:, 3, f:f + 1]
                z = zpool.tile([128, NB], fp32, tag="z")
                nc.vector.tensor_scalar(z[:], ps[:], m, g,
                                        op0=mybir.AluOpType.subtract,
                                        op1=mybir.AluOpType.mult)
                y = zpool.tile([128, NB], fp32, tag="y")
                nc.vector.tensor_scalar(y[:], z[:], r, b,
                                        op0=mybir.AluOpType.mult,
                                        op1=mybir.AluOpType.add)
                if li == 3:
                    nc.sync.dma_start(out=out[f * 128:(f + 1) * 128, :], in_=y[:])
                    return
                t = zpool.tile([128, NB], fp8, tag="t")
                nc.vector.tensor_scalar(t[:], y[:], 0.0, None,
                                        op0=mybir.AluOpType.is_ge)
                nc.scalar.activation(htile[:, f, :], t[:],
                                     mybir.ActivationFunctionType.Copy,
                                     bias=-1.0, scale=2.0)

            # ---- Layer 1: two fp32r passes (lo, hi) per contiguous K chunk,
            #      ascending, one PSUM accumulation group ----
            for f in range(F1):
                if f == 0:
                    w8 = w8_f0
                else:
                    w8 = w8pool.tile([128, K1, 128], fp8, tag="w18")
                    nc.sync.dma_start(out=w8[:], in_=w1[f])
                wf = w32pool.tile([128, K1, 128], fp32, tag="w1f")
                if f == 0:
                    # per-chunk converts so the first matmul issues ~2.5us in
                    for k in range(K1):
                        nc.gpsimd.tensor_copy(wf[:, k, :].bitcast(fp32r),
                                              w8[:, k, :])
                else:
                    nc.gpsimd.tensor_copy(wf[:].bitcast(fp32r), w8[:])
                ps = psum.tile([128, NB], fp32, tag="ps")
                for k in range(K1):
                    nc.tensor.matmul(ps[:], wf[:, k, :].bitcast(fp32r),
                                     xl[:, k, :].bitcast(fp32r),
                                     start=(k == 0), stop=False)
                    nc.tensor.matmul(ps[:], wf[:, k, :].bitcast(fp32r),
                                     xh[:, k, :],
                                     start=False, stop=(k == K1 - 1))
                epilogue(ps, 0, f, hs[0], F1)

            # Prefetch the first two w4 tiles during layer 2 so layer 4 does
            # not start DMA-starved.
            w4pre = []
            for f in range(2):
                w8 = persist.tile([128, K2, 128], fp8, tag=f"w4p{f}",
                                  name=f"w4p{f}")
                nc.sync.dma_start(out=w8[:], in_=w4[f])
                w4pre.append(w8)

            # ---- Layers 2-3: fp8 DoubleRow (exact) ----
            for li, (w, hin, hout) in enumerate([(w2, hs[0], hs[1]),
                                                 (w3, hs[1], hs[2])], start=1):
                for f in range(F1):
                    w8 = w8pool.tile([128, K2, 128], fp8, tag="w8")
                    nc.sync.dma_start(out=w8[:], in_=w[f])
                    ps = psum.tile([128, NB], fp32, tag="ps")
                    for i in range(K2 // 2):
                        nc.tensor.matmul(ps[:], w8[:, 2 * i:2 * i + 2, :],
                                         hin[:, 2 * i:2 * i + 2, :],
                                         start=(i == 0), stop=(i == K2 // 2 - 1),
                                         perf_mode=DR)
                    epilogue(ps, li, f, hout, F1)

            # ---- Layer 4: fp8 DoubleRow + affine only ----
            for f in range(F4):
                if f < 2:
                    w8 = w4pre[f]
                else:
                    w8 = w8pool.tile([128, K2, 128], fp8, tag="w8")
                    nc.sync.dma_start(out=w8[:], in_=w4[f])
                ps = psum.tile([128, NB], fp32, tag="ps")
                for i in range(K2 // 2):
                    nc.tensor.matmul(ps[:], w8[:, 2 * i:2 * i + 2, :],
                                     hs[2][:, 2 * i:2 * i + 2, :],
                                     start=(i == 0), stop=(i == K2 // 2 - 1),
                                     perf_mode=DR)
                epilogue(ps, 3, f, None, F4)

    nc.finalize()
    return nc


def _device_rsqrt(v):
    """rsqrt(v + eps) with the same bits as the neuron reference (GPSIMD op)."""
    import jax
    fn = _CACHE.get("rsqrt_fn")
    if fn is None:
        fn = jax.jit(lambda t: jax.lax.rsqrt(t + EPS))
        _CACHE["rsqrt_fn"] = fn
    return np.asarray(fn(v.astype(np.float32)))


def _sign8(w):
    return np.where(w >= 0, 1, -1).astype(ml_dtypes.float8_e4m3)


def _prep_w(ws, n_k, n_f):
    # ws: [F_total, K_total] +-1 fp8 -> [n_f, 128, n_k, 128]:
    # out[f, p, k, j] = ws[f*128+j, k*128+p]
    Ft, Kt = ws.shape
    a = ws.reshape(n_f, 128, n_k, 128)          # [f, j, k, p]
    return np.ascontiguousarray(a.transpose(0, 3, 2, 1))


def _prep_bn(bn, n_f):
    # bn: [4, F] (g, b, m, v) -> [128, 4, n_f] with rows (m, g, r, b)
    g, b, m, v = bn[0], bn[1], bn[2], bn[3]
    r = _device_rsqrt(v)
    stack = np.stack([m, g, r, b])              # [4, F]
    a = stack.reshape(4, n_f, 128)              # [c, f, p]
    return np.ascontiguousarray(a.transpose(2, 0, 1)).astype(np.float32)


def kernel(x, w1, w2, w3, w4, bn1, bn2, bn3, bn4):
    from concourse.bass_utils import run_bass_kernel_spmd

    x = np.asarray(x, dtype=np.float32)
    nc = _CACHE.get("nc")
    if nc is None:
        nc = _build_bass()
        _CACHE["nc"] = nc

    w1p = _prep_w(_sign8(np.asarray(w1)), K1, F1)
    w2p = _prep_w(_sign8(np.asarray(w2)), K2, F1)
    w3p = _prep_w(_sign8(np.asarray(w3)), K2, F1)
    w4s = _sign8(np.asarray(w4))
    w4pad = np.zeros((OUTP, H), dtype=ml_dtypes.float8_e4m3)
    w4pad[:OUT] = w4s
    w4p = _prep_w(w4pad, K2, F4)

    b123 = np.stack([_prep_bn(np.asarray(bn1), F1),
                     _prep_bn(np.asarray(bn2), F1),
                     _prep_bn(np.asarray(bn3), F1)], axis=1)  # [128, 3, 4, F1]
    b123 = np.ascontiguousarray(b123)
    bn4pad = np.zeros((4, OUTP), dtype=np.float32)
    bn4pad[:, :OUT] = np.asarray(bn4)
    bn4pad[3, OUT:] = 1.0
    b4 = _prep_bn(bn4pad, F4)

    in_maps = []
    for c in range(NCORES):
        xs = x[c * NB:(c + 1) * NB]                     # [512, 3072]
        # xT layout [128, K1, NB]: [p, k, n] = x[n, k*128+p]
        xt = np.ascontiguousarray(
            xs.reshape(NB, K1, 128).transpose(2, 1, 0))
        in_maps.append({"xT": xt, "w1": w1p, "w2": w2p, "w3": w3p,
                        "w4": w4p, "bn123": b123, "bn4": b4})

    import os
    trace = bool(os.environ.get("BNN_TRACE"))
    res = run_bass_kernel_spmd(nc, in_maps, core_ids=list(range(NCORES)),
                               trace=trace)
    if trace:
        _CACHE["last_exec_time_ns"] = res.exec_time_ns
        _CACHE["last_profile"] = res.profile_json

    # Gather: out [OUTP, NB] feature-major -> [B, OUT]
    y = np.empty((B, OUT), dtype=np.float32)
    for c in range(NCORES):
        y[c * NB:(c + 1) * NB] = res.results[c]["out"][:OUT, :].T

    _CACHE["last_y"] = y
    # npasses tail, replicating XLA's CSE'd graph bitwise:
    acc = y + y
    acc = acc + y
    acc = acc + y
    return acc * np.float32(0.25)

